# revision 38
# baseline (speedup 1.0000x reference)
"""HalfEdgeConv Trainium2 kernel.

out[e] = relu(W @ concat(x[next_idx[e]], has_twin[e] ? x[twin_idx[e]] : 0) + b)

Strategy (data-parallel over half-edges, 8 cores):
  - x cast to bf16 and stored as [N+1, 128] rows: 64 features + 64 zeros
    (row N all-zero). 256B row stride suits dma_gather; the zero half means
    a tile with no live twins needs no twin data at all.
  - Per core, edges are bucketed by next_idx>>15 (31 buckets of <=32768
    rows so indices fit int16) and sorted live-twin-first inside each
    bucket. Each bucket's next-features come from ONE dma_gather (4480
    rows/instruction) instead of one indirect DMA per 128 edges — this
    removes the ~1us/instruction SWDGE serialization that dominated the
    baseline.
  - Twin features (int32 global indices, dead twins -> zero row N) use one
    per-tile indirect DMA, but only for the first 18 tiles of each bucket
    where the host packed all live-twin edges.
  - Compute per tile: PE transposes the gathered [128 edges, 128ch] tile
    (next-transpose covers all 128 channel rows incl. the zero half; the
    twin transpose then overwrites channel rows 64..127), DVE copies
    PSUM->SBUF, one PE matmul per 4 tiles with stationary W.T, and one ACT
    instruction applies per-partition bias + ReLU writing bf16.
  - Output is channel-major [64, slots]; the host transposes, casts to f32
    and scatters slots back to edge order.
"""
import os
import sys

sys.path.insert(0, "/opt/trn_rl_repo")

import numpy as np
import ml_dtypes
from contextlib import ExitStack

import concourse.bass as bass
import concourse.tile as tile
from concourse import bacc, mybir, bass_utils

N = 1_000_000
C = 64
NCORES = 8
P = 128
EPC = 126976                # edges per core (baseline slicing)
NPAD = NCORES * EPC         # 1015808 padded edges
BK = 32768                  # x rows per bucket (int16 index range)
NB = 31                     # buckets (N+1 rows)
TBK = 34                    # tile slots per bucket  (cap 4352 edges)
LT = 20                     # live-twin tiles per bucket (cap 2560 live)
TILES = NB * TBK            # 1085 tiles per core
SLOTS = TILES * P           # 138880 slots per core
NIF = TBK * P // 16         # 280: idx free dim per bucket
G = 4                       # tiles per PSUM group

f32 = mybir.dt.float32
bf16 = mybir.dt.bfloat16
i32 = mybir.dt.int32
i16 = mybir.dt.int16
bfnp = ml_dtypes.bfloat16

_COMPILED = None
LAST_EXEC_NS = None

# Next-side gather strategy: rows per dma_gather instruction, or 0 to use
# one indirect DMA per 128-edge tile. Indirect is ~8.6ns/row vs gather's
# ~9.0ns/row on the Q7 generator, but all-indirect triples the instruction
# count and the Pool sequencer's ~350ns/instruction dispatch overhead then
# dominates (measured 2.31ms vs 2.13ms) — so big gathers win.
GATHER_NIDX = int(os.environ.get("KN_GATHER_NIDX", "512")) or None


def _try_install_ntff_shim():
    """NTFF profiling hook (trace runs only); degrade silently if absent."""
    import types, ctypes, contextlib
    if "antenv.axon_hooks" in sys.modules:
        return
    try:
        import antenv
        mod = types.ModuleType("antenv.axon_hooks")
        mod._hook = None
        mod.set_axon_ntff_profile_hook = lambda h: setattr(mod, "_hook", h)
        mod.get_axon_ntff_profile_hook = lambda: mod._hook
        sys.modules["antenv.axon_hooks"] = mod
        antenv.axon_hooks = mod
        lib = ctypes.CDLL("/opt/axon/libaxon_pjrt.so")
        if not hasattr(lib, "axon_start_nrt_profile"):
            return
        lib.axon_start_nrt_profile.argtypes = [ctypes.POINTER(ctypes.c_int64), ctypes.c_size_t]
        lib.axon_start_nrt_profile.restype = ctypes.c_int64
        lib.axon_stop_nrt_profile.argtypes = [ctypes.c_char_p]
        lib.axon_stop_nrt_profile.restype = ctypes.c_int64

        @contextlib.contextmanager
        def _hook(output_dir, device_ids):
            import jax
            jax.devices()
            if device_ids:
                ids = (ctypes.c_int64 * len(device_ids))(*device_ids)
                rc = lib.axon_start_nrt_profile(ids, len(device_ids))
            else:
                rc = lib.axon_start_nrt_profile(None, 0)
            if rc != 0:
                raise RuntimeError(f"axon_start_nrt_profile rc={rc}")
            try:
                yield
            finally:
                lib.axon_stop_nrt_profile(str(output_dir).encode())

        mod.set_axon_ntff_profile_hook(_hook)
    except Exception:
        pass


# Per-bucket SWDGE emission: 20 [indirect, gather] pairs = 40 instructions,
# a multiple of the 8 DMASW semaphore lanes, so the scheduler's steady-state
# interleave keeps each lane type-pure. A first build observes the actual
# scheduled order; gathers landing on lanes that never carry a (queue-0
# pinned) indirect are reassigned to SWDGE queue 1 so their Q7 descriptor
# generation runs on pair 1, concurrent with pair 0.
GSIZES = [256] * 14 + [128] * 6         # 20 chunks = 4352 rows per bucket


def _build(queue_map=None, record=None):
    nc = bacc.Bacc("TRN2", target_bir_lowering=False, debug=False,
                   num_swdge_queues=4)
    x_d = nc.dram_tensor("x2", [N + 1, P], bf16, kind="ExternalInput").ap()
    nix_d = nc.dram_tensor("nix", [P, NB * NIF], i16, kind="ExternalInput").ap()
    nix32_d = nc.dram_tensor("nix32", [P, TILES], i32, kind="ExternalInput").ap()
    tix_d = nc.dram_tensor("tix", [P, NB * LT], i32, kind="ExternalInput").ap()
    wt_d = nc.dram_tensor("wt", [2 * C, C], bf16, kind="ExternalInput").ap()
    b_d = nc.dram_tensor("bias", [C, 1], f32, kind="ExternalInput").ap()
    id_d = nc.dram_tensor("ident", [P, P], bf16, kind="ExternalInput").ap()
    out_d = nc.dram_tensor("out", [C, SLOTS], bf16, kind="ExternalOutput").ap()

    NGRP = (TBK + G - 1) // G

    with tile.TileContext(nc) as tc:
        with ExitStack() as ctx:
            const = ctx.enter_context(tc.tile_pool(name="const", bufs=1))
            catp = ctx.enter_context(tc.tile_pool(name="catp", bufs=3))
            ctwp = ctx.enter_context(tc.tile_pool(name="ctwp", bufs=3))
            ctp = ctx.enter_context(tc.tile_pool(name="ctp", bufs=3))
            outp = ctx.enter_context(tc.tile_pool(name="outp", bufs=2))
            ptp = ctx.enter_context(tc.tile_pool(name="ptp", bufs=2, space="PSUM"))
            pop = ctx.enter_context(tc.tile_pool(name="pop", bufs=3, space="PSUM"))

            wt_sb = const.tile([2 * C, C], bf16)
            nc.sync.dma_start(wt_sb[:], wt_d[:])
            b_sb = const.tile([C, 1], f32)
            nc.sync.dma_start(b_sb[:], b_d[:])
            id_sb = const.tile([P, P], bf16)
            nc.sync.dma_start(id_sb[:], id_d[:])
            if GATHER_NIDX:
                nix_sb = const.tile([P, NB * NIF], i16)
                nc.sync.dma_start(nix_sb[:], nix_d[:])
            else:
                nix32_sb = const.tile([P, TILES], i32)
                nc.sync.dma_start(nix32_sb[:], nix32_d[:])
            tix_sb = const.tile([P, NB * LT], i32)
            nc.sync.dma_start(tix_sb[:], tix_d[:])

            swdge_pos = 0
            for b in range(NB):
                rows = min(BK, N + 1 - b * BK)
                catN = catp.tile([P, TBK, P], bf16, tag="catN")
                catT = ctwp.tile([P, LT, P], bf16, tag="catT")
                if GATHER_NIDX:
                    assert sum(GSIZES) == TBK * P and len(GSIZES) == LT
                    col = 0
                    row0 = 0
                    for k in range(LT):
                        # even DMASW lane: twin indirect (pinned to queue 0)
                        j = b * LT + k
                        nc.gpsimd.indirect_dma_start(
                            out=catT[:, k, :], out_offset=None, in_=x_d[:],
                            in_offset=bass.IndirectOffsetOnAxis(
                                ap=tix_sb[:, j:j + 1], axis=0))
                        s = GSIZES[k]
                        gi = nc.gpsimd.dma_gather(
                            out_ap=catN[:, row0:row0 + s // P, :],
                            in_ap=x_d[b * BK:b * BK + rows, :],
                            idxs_ap=nix_sb[:, b * NIF + col:
                                           b * NIF + col + s // 16],
                            num_idxs=s,
                            num_idxs_reg=s,
                            elem_size=P,
                            queue_num=(queue_map or {}).get((b, k), 0))
                        if record is not None:
                            record[gi.ins.name] = (b, k)
                        col += s // 16
                        row0 += s // P
                else:
                    for k in range(TBK):
                        t = b * TBK + k
                        nc.gpsimd.indirect_dma_start(
                            out=catN[:, k, :], out_offset=None, in_=x_d[:],
                            in_offset=bass.IndirectOffsetOnAxis(
                                ap=nix32_sb[:, t:t + 1], axis=0))
                    for k in range(LT):
                        j = b * LT + k
                        nc.gpsimd.indirect_dma_start(
                            out=catT[:, k, :], out_offset=None, in_=x_d[:],
                            in_offset=bass.IndirectOffsetOnAxis(
                                ap=tix_sb[:, j:j + 1], axis=0))

                ot = outp.tile([C, TBK * P], bf16, tag="ot")
                for g in range(NGRP):
                    gt = min(G, TBK - g * G)
                    pt = ptp.tile([P, G * P], bf16, tag="pt")
                    for kk in range(gt):
                        t = g * G + kk
                        nc.tensor.transpose(
                            out=pt[:, kk * P:(kk + 1) * P],
                            in_=catN[:, t, :], identity=id_sb[:])
                        if t < LT:
                            nc.tensor.transpose(
                                out=pt[C:P, kk * P:(kk + 1) * P],
                                in_=catT[:, t, 0:C], identity=id_sb[:])
                    ct = ctp.tile([P, G * P], bf16, tag="ct")
                    nc.vector.tensor_copy(ct[:, :gt * P], pt[:, :gt * P])
                    po = pop.tile([C, G * P], f32, tag="po")
                    nc.tensor.matmul(out=po[:, :gt * P], lhsT=wt_sb[:],
                                     rhs=ct[:, :gt * P], start=True, stop=True)
                    nc.scalar.activation(
                        ot[:, g * G * P:g * G * P + gt * P], po[:, :gt * P],
                        mybir.ActivationFunctionType.Relu,
                        bias=b_sb[:, 0:1])
                nc.sync.dma_start(
                    out_d[:, b * TBK * P:(b + 1) * TBK * P], ot[:])

    nc.compile()
    return nc


def _get_compiled():
    global _COMPILED
    if _COMPILED is None:
        if GATHER_NIDX:
            # Pass 1: observe the scheduled SWDGE order; find DMASW lanes
            # that never carry an indirect (pinned to queue 0) and move the
            # gathers on those lanes to queue 1 (Q7 pair 1 runs their
            # descriptor generation concurrently with pair 0).
            record = {}
            nc1 = _build(record=record)
            seq = []
            for blk in nc1.m.functions[0].blocks:
                for inst in blk.instructions:
                    if isinstance(inst, mybir.InstDMAGatherAnt):
                        seq.append(("g", inst.name))
                    elif (isinstance(inst, mybir.InstDMACopy)
                          and inst.engine == mybir.EngineType.Pool):
                        seq.append(("i", None))
            lane_has_i = {k % 8 for k, (t, _) in enumerate(seq) if t == "i"}
            qmap = {}
            for k, (t, nm) in enumerate(seq):
                if t == "g" and (k % 8) not in lane_has_i and nm in record:
                    qmap[record[nm]] = 1
            _COMPILED = _build(queue_map=qmap)
        else:
            _COMPILED = _build()
    return _COMPILED


def _prep_core(nloc, tloc, eid_base):
    """Bucket/sort one core's edges; build device index tables.

    nloc/tloc: [EPC] int32 next / twin(redirected, N=dead) indices.
    Returns (nix [128, NB*NIF] i16, tix [128, NB*LT] i32, eid [SLOTS] i64).
    """
    live = tloc != N
    nb = nloc >> 15
    order = np.lexsort((~live, nb))
    sn = nloc[order]
    st = tloc[order]
    seid = eid_base + order.astype(np.int64)
    counts = np.bincount(nb, minlength=NB)

    nix = np.zeros((P, NB * NIF), np.int16)
    nix32 = np.zeros((P, TILES), np.int32)
    tix = np.full((P, NB * LT), N, np.int32)
    eid = np.full(SLOTS, -1, np.int64)

    off = 0
    for b in range(NB):
        cnt = int(counts[b])
        if cnt > TBK * P:
            raise RuntimeError(f"bucket {b} overflow: {cnt} > {TBK * P}")
        nlive = int(live[order[off:off + cnt]].sum())
        if nlive > LT * P:
            raise RuntimeError(f"bucket {b} live overflow: {nlive} > {LT * P}")
        reqs = np.zeros(TBK * P, np.int32)
        reqs[:cnt] = sn[off:off + cnt] - (b << 15)
        a = reqs.reshape(NIF, 16).T.astype(np.int16)      # [16, NIF]
        for r in range(0, P, 16):   # replicate for every Q7 pair (queues 0-3)
            nix[r:r + 16, b * NIF:(b + 1) * NIF] = a
        gq = np.zeros(TBK * P, np.int32)
        gq[:cnt] = sn[off:off + cnt]
        nix32[:, b * TBK:(b + 1) * TBK] = gq.reshape(TBK, P).T
        tw = np.full(LT * P, N, np.int32)
        m = min(cnt, LT * P)
        tw[:m] = st[off:off + m]
        tix[:, b * LT:(b + 1) * LT] = tw.reshape(LT, P).T
        eid[b * TBK * P:b * TBK * P + cnt] = seid[off:off + cnt]
        off += cnt
    return nix, nix32, tix, eid


def kernel(x, next_idx, twin_idx, has_twin, W, b):
    global LAST_EXEC_NS
    x = np.asarray(x, dtype=np.float32)
    next_idx = np.asarray(next_idx, dtype=np.int32)
    twin_idx = np.asarray(twin_idx, dtype=np.int32)
    has_twin = np.asarray(has_twin)
    W = np.asarray(W, dtype=np.float32)
    b = np.asarray(b, dtype=np.float32)

    trace = bool(os.environ.get("BASS_TRACE"))
    if trace:
        _try_install_ntff_shim()

    # x table: [N+1, 128] bf16 rows = 64 features + 64 zeros; row N all-zero.
    x2 = np.zeros((N + 1, P), bfnp)
    x2[:N, :C] = x.astype(bfnp)
    npad = np.zeros(NPAD, np.int32)
    npad[:N] = next_idx
    npad[N:] = (np.arange(NPAD - N, dtype=np.int64) * 65537 % N).astype(np.int32)
    tpad = np.full(NPAD, N, np.int32)
    tpad[:N] = np.where(has_twin, twin_idx, N).astype(np.int32)

    wt = np.ascontiguousarray(W.T).astype(bfnp)         # [128, 64]
    bias = np.ascontiguousarray(b.reshape(C, 1))        # [64, 1] f32
    ident = np.eye(P, dtype=np.float32).astype(bfnp)

    in_maps = []
    eids = []
    for c in range(NCORES):
        sl = slice(c * EPC, (c + 1) * EPC)
        nix, nix32, tix, eid = _prep_core(npad[sl], tpad[sl], c * EPC)
        eids.append(eid)
        in_maps.append({"x2": x2, "nix": nix, "nix32": nix32, "tix": tix,
                        "wt": wt, "bias": bias, "ident": ident})

    nc = _get_compiled()
    res = bass_utils.run_bass_kernel_spmd(
        nc, in_maps, core_ids=list(range(NCORES)), trace=trace)
    LAST_EXEC_NS = res.exec_time_ns

    out = np.empty((N, C), np.float32)
    for c in range(NCORES):
        arr = res.results[c]["out"].T.astype(np.float32)   # [SLOTS, 64]
        eid = eids[c]
        m = (eid >= 0) & (eid < N)
        out[eid[m]] = arr[m]
    return out


# revision 44
# speedup vs baseline: 1.1126x; 1.1126x over previous
"""HalfEdgeConv Trainium2 kernel.

out[e] = relu(W @ concat(x[next_idx[e]], has_twin[e] ? x[twin_idx[e]] : 0) + b)

Strategy (data-parallel over half-edges, 8 cores):
  - x cast to bf16 and stored as [N+1, 128] rows: 64 features + 64 zeros
    (row N all-zero). 256B row stride suits dma_gather; the zero half means
    a tile with no live twins needs no twin data at all.
  - Per core, edges are bucketed by next_idx>>15 (31 buckets of <=32768
    rows so indices fit int16) and sorted live-twin-first inside each
    bucket. Each bucket's next-features come from ONE dma_gather (4480
    rows/instruction) instead of one indirect DMA per 128 edges — this
    removes the ~1us/instruction SWDGE serialization that dominated the
    baseline.
  - Twin features (int32 global indices, dead twins -> zero row N) use one
    per-tile indirect DMA, but only for the first 18 tiles of each bucket
    where the host packed all live-twin edges.
  - Compute per tile: PE transposes the gathered [128 edges, 128ch] tile
    (next-transpose covers all 128 channel rows incl. the zero half; the
    twin transpose then overwrites channel rows 64..127), DVE copies
    PSUM->SBUF, one PE matmul per 4 tiles with stationary W.T, and one ACT
    instruction applies per-partition bias + ReLU writing bf16.
  - Output is channel-major [64, slots]; the host transposes, casts to f32
    and scatters slots back to edge order.
"""
import os
import sys

sys.path.insert(0, "/opt/trn_rl_repo")

import numpy as np
import ml_dtypes
from contextlib import ExitStack

import concourse.bass as bass
import concourse.tile as tile
from concourse import bacc, mybir, bass_utils

N = 1_000_000
C = 64
NCORES = 8
P = 128
EPC = 126976                # edges per core (baseline slicing)
NPAD = NCORES * EPC         # 1015808 padded edges
BK = 32768                  # x rows per bucket (int16 index range)
NB = 31                     # buckets (N+1 rows)
TBK = 34                    # tile slots per bucket  (cap 4352 edges)
LT = 18                     # live-twin tiles per bucket (cap 2304 live)
TILES = NB * TBK            # 1085 tiles per core
SLOTS = TILES * P           # 138880 slots per core
NIF = TBK * P // 16         # 280: idx free dim per bucket
G = 4                       # tiles per PSUM group

f32 = mybir.dt.float32
bf16 = mybir.dt.bfloat16
i32 = mybir.dt.int32
i16 = mybir.dt.int16
bfnp = ml_dtypes.bfloat16

_COMPILED = None
LAST_EXEC_NS = None

# Next-side gather strategy: rows per dma_gather instruction, or 0 to use
# one indirect DMA per 128-edge tile. Indirect is ~8.6ns/row vs gather's
# ~9.0ns/row on the Q7 generator, but all-indirect triples the instruction
# count and the Pool sequencer's ~350ns/instruction dispatch overhead then
# dominates (measured 2.31ms vs 2.13ms) — so big gathers win.
GATHER_NIDX = int(os.environ.get("KN_GATHER_NIDX", "512")) or None


def _try_install_ntff_shim():
    """NTFF profiling hook (trace runs only); degrade silently if absent."""
    import types, ctypes, contextlib
    if "antenv.axon_hooks" in sys.modules:
        return
    try:
        import antenv
        mod = types.ModuleType("antenv.axon_hooks")
        mod._hook = None
        mod.set_axon_ntff_profile_hook = lambda h: setattr(mod, "_hook", h)
        mod.get_axon_ntff_profile_hook = lambda: mod._hook
        sys.modules["antenv.axon_hooks"] = mod
        antenv.axon_hooks = mod
        lib = ctypes.CDLL("/opt/axon/libaxon_pjrt.so")
        if not hasattr(lib, "axon_start_nrt_profile"):
            return
        lib.axon_start_nrt_profile.argtypes = [ctypes.POINTER(ctypes.c_int64), ctypes.c_size_t]
        lib.axon_start_nrt_profile.restype = ctypes.c_int64
        lib.axon_stop_nrt_profile.argtypes = [ctypes.c_char_p]
        lib.axon_stop_nrt_profile.restype = ctypes.c_int64

        @contextlib.contextmanager
        def _hook(output_dir, device_ids):
            import jax
            jax.devices()
            if device_ids:
                ids = (ctypes.c_int64 * len(device_ids))(*device_ids)
                rc = lib.axon_start_nrt_profile(ids, len(device_ids))
            else:
                rc = lib.axon_start_nrt_profile(None, 0)
            if rc != 0:
                raise RuntimeError(f"axon_start_nrt_profile rc={rc}")
            try:
                yield
            finally:
                lib.axon_stop_nrt_profile(str(output_dir).encode())

        mod.set_axon_ntff_profile_hook(_hook)
    except Exception:
        pass


def _build(gather_rows=None):
    # gather_rows[b]: requests to fetch for bucket b (multiple of 128,
    # <= TBK*P), derived from the actual inputs' per-bucket maximum across
    # cores so mostly-empty buckets don't fetch padding rows.
    if gather_rows is None:
        gather_rows = [TBK * P] * NB
    nc = bacc.Bacc("TRN2", target_bir_lowering=False, debug=False)
    x_d = nc.dram_tensor("x2", [N + 1, P], bf16, kind="ExternalInput").ap()
    nix_d = nc.dram_tensor("nix", [P, NB * NIF], i16, kind="ExternalInput").ap()
    nix32_d = nc.dram_tensor("nix32", [P, TILES], i32, kind="ExternalInput").ap()
    tix_d = nc.dram_tensor("tix", [P, NB * LT], i32, kind="ExternalInput").ap()
    wt_d = nc.dram_tensor("wt", [2 * C, C], bf16, kind="ExternalInput").ap()
    b_d = nc.dram_tensor("bias", [C, 1], f32, kind="ExternalInput").ap()
    id_d = nc.dram_tensor("ident", [P, P], bf16, kind="ExternalInput").ap()
    out_d = nc.dram_tensor("out", [C, SLOTS], bf16, kind="ExternalOutput").ap()

    NGRP = (TBK + G - 1) // G

    with tile.TileContext(nc) as tc:
        with ExitStack() as ctx:
            const = ctx.enter_context(tc.tile_pool(name="const", bufs=1))
            catp = ctx.enter_context(tc.tile_pool(name="catp", bufs=3))
            ctwp = ctx.enter_context(tc.tile_pool(name="ctwp", bufs=3))
            ctp = ctx.enter_context(tc.tile_pool(name="ctp", bufs=3))
            outp = ctx.enter_context(tc.tile_pool(name="outp", bufs=2))
            ptp = ctx.enter_context(tc.tile_pool(name="ptp", bufs=2, space="PSUM"))
            pop = ctx.enter_context(tc.tile_pool(name="pop", bufs=3, space="PSUM"))

            wt_sb = const.tile([2 * C, C], bf16)
            nc.sync.dma_start(wt_sb[:], wt_d[:])
            b_sb = const.tile([C, 1], f32)
            nc.sync.dma_start(b_sb[:], b_d[:])
            id_sb = const.tile([P, P], bf16)
            nc.sync.dma_start(id_sb[:], id_d[:])
            if GATHER_NIDX:
                nix_sb = const.tile([P, NB * NIF], i16)
                nc.sync.dma_start(nix_sb[:], nix_d[:])
            else:
                nix32_sb = const.tile([P, TILES], i32)
                nc.sync.dma_start(nix32_sb[:], nix32_d[:])
            tix_sb = const.tile([P, NB * LT], i32)
            nc.sync.dma_start(tix_sb[:], tix_d[:])

            swdge_pos = 0
            for b in range(NB):
                rows = min(BK, N + 1 - b * BK)
                catN = catp.tile([P, TBK, P], bf16, tag="catN")
                catT = ctwp.tile([P, LT, P], bf16, tag="catT")
                if GATHER_NIDX:
                    rem = gather_rows[b]
                    col = 0
                    row0 = 0
                    while rem > 0:
                        s = min(GATHER_NIDX, rem)
                        assert s % P == 0
                        nc.gpsimd.dma_gather(
                            out_ap=catN[:, row0:row0 + s // P, :],
                            in_ap=x_d[b * BK:b * BK + rows, :],
                            idxs_ap=nix_sb[:, b * NIF + col:
                                           b * NIF + col + s // 16],
                            num_idxs=s,
                            num_idxs_reg=s,
                            elem_size=P)
                        rem -= s
                        col += s // 16
                        row0 += s // P
                    for k in range(LT):
                        j = b * LT + k
                        nc.gpsimd.indirect_dma_start(
                            out=catT[:, k, :], out_offset=None, in_=x_d[:],
                            in_offset=bass.IndirectOffsetOnAxis(
                                ap=tix_sb[:, j:j + 1], axis=0))
                else:
                    for k in range(TBK):
                        t = b * TBK + k
                        nc.gpsimd.indirect_dma_start(
                            out=catN[:, k, :], out_offset=None, in_=x_d[:],
                            in_offset=bass.IndirectOffsetOnAxis(
                                ap=nix32_sb[:, t:t + 1], axis=0))
                    for k in range(LT):
                        j = b * LT + k
                        nc.gpsimd.indirect_dma_start(
                            out=catT[:, k, :], out_offset=None, in_=x_d[:],
                            in_offset=bass.IndirectOffsetOnAxis(
                                ap=tix_sb[:, j:j + 1], axis=0))

                ot = outp.tile([C, TBK * P], bf16, tag="ot")
                for g in range(NGRP):
                    gt = min(G, TBK - g * G)
                    pt = ptp.tile([P, G * P], bf16, tag="pt")
                    for kk in range(gt):
                        t = g * G + kk
                        nc.tensor.transpose(
                            out=pt[:, kk * P:(kk + 1) * P],
                            in_=catN[:, t, :], identity=id_sb[:])
                        if t < LT:
                            nc.tensor.transpose(
                                out=pt[C:P, kk * P:(kk + 1) * P],
                                in_=catT[:, t, 0:C], identity=id_sb[:])
                    ct = ctp.tile([P, G * P], bf16, tag="ct")
                    nc.vector.tensor_copy(ct[:, :gt * P], pt[:, :gt * P])
                    po = pop.tile([C, G * P], f32, tag="po")
                    nc.tensor.matmul(out=po[:, :gt * P], lhsT=wt_sb[:],
                                     rhs=ct[:, :gt * P], start=True, stop=True)
                    nc.scalar.activation(
                        ot[:, g * G * P:g * G * P + gt * P], po[:, :gt * P],
                        mybir.ActivationFunctionType.Relu,
                        bias=b_sb[:, 0:1])
                nc.sync.dma_start(
                    out_d[:, b * TBK * P:(b + 1) * TBK * P], ot[:])

    nc.compile()
    return nc


def _get_compiled(gather_rows=None):
    global _COMPILED
    if _COMPILED is None:
        _COMPILED = _build(gather_rows)
    return _COMPILED


def _prep_core(nloc, tloc, eid_base):
    """Bucket/sort one core's edges; build device index tables.

    nloc/tloc: [EPC] int32 next / twin(redirected, N=dead) indices.
    Returns (nix [128, NB*NIF] i16, tix [128, NB*LT] i32, eid [SLOTS] i64).
    """
    live = tloc != N
    nb = nloc >> 15
    order = np.lexsort((~live, nb))
    sn = nloc[order]
    st = tloc[order]
    seid = eid_base + order.astype(np.int64)
    counts = np.bincount(nb, minlength=NB)

    nix = np.zeros((P, NB * NIF), np.int16)
    nix32 = np.zeros((P, TILES), np.int32)
    tix = np.full((P, NB * LT), N, np.int32)
    eid = np.full(SLOTS, -1, np.int64)

    off = 0
    for b in range(NB):
        cnt = int(counts[b])
        if cnt > TBK * P:
            raise RuntimeError(f"bucket {b} overflow: {cnt} > {TBK * P}")
        nlive = int(live[order[off:off + cnt]].sum())
        if nlive > LT * P:
            raise RuntimeError(f"bucket {b} live overflow: {nlive} > {LT * P}")
        reqs = np.zeros(TBK * P, np.int32)
        reqs[:cnt] = sn[off:off + cnt] - (b << 15)
        a = reqs.reshape(NIF, 16).T.astype(np.int16)      # [16, NIF]
        for r in range(0, P, 16):   # replicate for every Q7 pair (queues 0-3)
            nix[r:r + 16, b * NIF:(b + 1) * NIF] = a
        gq = np.zeros(TBK * P, np.int32)
        gq[:cnt] = sn[off:off + cnt]
        nix32[:, b * TBK:(b + 1) * TBK] = gq.reshape(TBK, P).T
        tw = np.full(LT * P, N, np.int32)
        m = min(cnt, LT * P)
        tw[:m] = st[off:off + m]
        tix[:, b * LT:(b + 1) * LT] = tw.reshape(LT, P).T
        eid[b * TBK * P:b * TBK * P + cnt] = seid[off:off + cnt]
        off += cnt
    return nix, nix32, tix, eid, counts


def kernel(x, next_idx, twin_idx, has_twin, W, b):
    global LAST_EXEC_NS
    x = np.asarray(x, dtype=np.float32)
    next_idx = np.asarray(next_idx, dtype=np.int32)
    twin_idx = np.asarray(twin_idx, dtype=np.int32)
    has_twin = np.asarray(has_twin)
    W = np.asarray(W, dtype=np.float32)
    b = np.asarray(b, dtype=np.float32)

    trace = bool(os.environ.get("BASS_TRACE"))
    if trace:
        _try_install_ntff_shim()

    # x table: [N+1, 128] bf16 rows = 64 features + 64 zeros; row N all-zero.
    x2 = np.zeros((N + 1, P), bfnp)
    x2[:N, :C] = x.astype(bfnp)
    npad = np.zeros(NPAD, np.int32)
    npad[:N] = next_idx
    npad[N:] = (np.arange(NPAD - N, dtype=np.int64) * 65537 % N).astype(np.int32)
    tpad = np.full(NPAD, N, np.int32)
    tpad[:N] = np.where(has_twin, twin_idx, N).astype(np.int32)

    wt = np.ascontiguousarray(W.T).astype(bfnp)         # [128, 64]
    bias = np.ascontiguousarray(b.reshape(C, 1))        # [64, 1] f32
    ident = np.eye(P, dtype=np.float32).astype(bfnp)

    in_maps = []
    eids = []
    maxc = np.zeros(NB, np.int64)
    for c in range(NCORES):
        sl = slice(c * EPC, (c + 1) * EPC)
        nix, nix32, tix, eid, counts = _prep_core(npad[sl], tpad[sl], c * EPC)
        eids.append(eid)
        maxc = np.maximum(maxc, counts)
        in_maps.append({"x2": x2, "nix": nix, "nix32": nix32, "tix": tix,
                        "wt": wt, "bias": bias, "ident": ident})

    grows = [min(TBK * P, int(-(-m // P)) * P) for m in maxc]
    nc = _get_compiled(grows)
    res = bass_utils.run_bass_kernel_spmd(
        nc, in_maps, core_ids=list(range(NCORES)), trace=trace)
    LAST_EXEC_NS = res.exec_time_ns

    out = np.empty((N, C), np.float32)
    for c in range(NCORES):
        arr = res.results[c]["out"].T.astype(np.float32)   # [SLOTS, 64]
        eid = eids[c]
        m = (eid >= 0) & (eid < N)
        out[eid[m]] = arr[m]
    return out


# revision 52
# speedup vs baseline: 1.1373x; 1.0222x over previous
"""HalfEdgeConv Trainium2 kernel.

out[e] = relu(W @ concat(x[next_idx[e]], has_twin[e] ? x[twin_idx[e]] : 0) + b)

Strategy (data-parallel over half-edges, 8 cores):
  - x cast to bf16 and stored as [N+1, 128] rows: 64 features + 64 zeros
    (row N all-zero). 256B row stride suits dma_gather; the zero half means
    a tile with no live twins needs no twin data at all.
  - Per core, edges are bucketed by next_idx>>15 (31 buckets of <=32768
    rows so indices fit int16) and sorted live-twin-first inside each
    bucket. Each bucket's next-features come from ONE dma_gather (4480
    rows/instruction) instead of one indirect DMA per 128 edges — this
    removes the ~1us/instruction SWDGE serialization that dominated the
    baseline.
  - Twin features (int32 global indices, dead twins -> zero row N) use one
    per-tile indirect DMA, but only for the first 18 tiles of each bucket
    where the host packed all live-twin edges.
  - Compute per tile: PE transposes the gathered [128 edges, 128ch] tile
    (next-transpose covers all 128 channel rows incl. the zero half; the
    twin transpose then overwrites channel rows 64..127), DVE copies
    PSUM->SBUF, one PE matmul per 4 tiles with stationary W.T, and one ACT
    instruction applies per-partition bias + ReLU writing bf16.
  - Output is channel-major [64, slots]; the host transposes, casts to f32
    and scatters slots back to edge order.
"""
import os
import sys

sys.path.insert(0, "/opt/trn_rl_repo")

import numpy as np
import ml_dtypes
from contextlib import ExitStack

import concourse.bass as bass
import concourse.tile as tile
from concourse import bacc, mybir, bass_utils

N = 1_000_000
C = 64
NCORES = 8
P = 128
EPC = 126976                # edges per core (baseline slicing)
NPAD = NCORES * EPC         # 1015808 padded edges
BK = 32768                  # x rows per bucket (int16 index range)
NB = 31                     # buckets (N+1 rows)
TBK = 34                    # tile slots per bucket  (cap 4352 edges)
LT = 18                     # live-twin tiles per bucket (cap 2304 live)
TILES = NB * TBK            # 1085 tiles per core
SLOTS = TILES * P           # 138880 slots per core
NIF = TBK * P // 16         # 280: idx free dim per bucket
G = 4                       # tiles per PSUM group

f32 = mybir.dt.float32
bf16 = mybir.dt.bfloat16
i32 = mybir.dt.int32
i16 = mybir.dt.int16
bfnp = ml_dtypes.bfloat16

_COMPILED = None
LAST_EXEC_NS = None

# Next-side gather strategy: rows per dma_gather instruction, or 0 to use
# one indirect DMA per 128-edge tile. Indirect is ~8.6ns/row vs gather's
# ~9.0ns/row on the Q7 generator, but all-indirect triples the instruction
# count and the Pool sequencer's ~350ns/instruction dispatch overhead then
# dominates (measured 2.31ms vs 2.13ms) — so big gathers win.
GATHER_NIDX = int(os.environ.get("KN_GATHER_NIDX", "512")) or None


def _try_install_ntff_shim():
    """NTFF profiling hook (trace runs only); degrade silently if absent."""
    import types, ctypes, contextlib
    if "antenv.axon_hooks" in sys.modules:
        return
    try:
        import antenv
        mod = types.ModuleType("antenv.axon_hooks")
        mod._hook = None
        mod.set_axon_ntff_profile_hook = lambda h: setattr(mod, "_hook", h)
        mod.get_axon_ntff_profile_hook = lambda: mod._hook
        sys.modules["antenv.axon_hooks"] = mod
        antenv.axon_hooks = mod
        lib = ctypes.CDLL("/opt/axon/libaxon_pjrt.so")
        if not hasattr(lib, "axon_start_nrt_profile"):
            return
        lib.axon_start_nrt_profile.argtypes = [ctypes.POINTER(ctypes.c_int64), ctypes.c_size_t]
        lib.axon_start_nrt_profile.restype = ctypes.c_int64
        lib.axon_stop_nrt_profile.argtypes = [ctypes.c_char_p]
        lib.axon_stop_nrt_profile.restype = ctypes.c_int64

        @contextlib.contextmanager
        def _hook(output_dir, device_ids):
            import jax
            jax.devices()
            if device_ids:
                ids = (ctypes.c_int64 * len(device_ids))(*device_ids)
                rc = lib.axon_start_nrt_profile(ids, len(device_ids))
            else:
                rc = lib.axon_start_nrt_profile(None, 0)
            if rc != 0:
                raise RuntimeError(f"axon_start_nrt_profile rc={rc}")
            try:
                yield
            finally:
                lib.axon_stop_nrt_profile(str(output_dir).encode())

        mod.set_axon_ntff_profile_hook(_hook)
    except Exception:
        pass


def _build(gather_rows=None, lt_tiles=None):
    # gather_rows[b]: requests to fetch for bucket b (multiple of 128,
    # <= TBK*P); lt_tiles[b]: twin-indirect tiles for bucket b (<= LT).
    # Both derived from the actual inputs' per-bucket maxima across cores
    # so mostly-empty buckets don't fetch padding rows.
    if gather_rows is None:
        gather_rows = [TBK * P] * NB
    if lt_tiles is None:
        lt_tiles = [LT] * NB
    nc = bacc.Bacc("TRN2", target_bir_lowering=False, debug=False)
    x_d = nc.dram_tensor("x2", [N + 1, P], bf16, kind="ExternalInput").ap()
    nix_d = nc.dram_tensor("nix", [P, NB * NIF], i16, kind="ExternalInput").ap()
    nix32_d = nc.dram_tensor("nix32", [P, TILES], i32, kind="ExternalInput").ap()
    tix_d = nc.dram_tensor("tix", [P, NB * LT], i32, kind="ExternalInput").ap()
    wt_d = nc.dram_tensor("wt", [2 * C, C], bf16, kind="ExternalInput").ap()
    b_d = nc.dram_tensor("bias", [C, 1], f32, kind="ExternalInput").ap()
    id_d = nc.dram_tensor("ident", [P, P], bf16, kind="ExternalInput").ap()
    out_d = nc.dram_tensor("out", [C, SLOTS], bf16, kind="ExternalOutput").ap()

    NGRP = (TBK + G - 1) // G

    with tile.TileContext(nc) as tc:
        with ExitStack() as ctx:
            const = ctx.enter_context(tc.tile_pool(name="const", bufs=1))
            catp = ctx.enter_context(tc.tile_pool(name="catp", bufs=3))
            ctwp = ctx.enter_context(tc.tile_pool(name="ctwp", bufs=3))
            ctp = ctx.enter_context(tc.tile_pool(name="ctp", bufs=3))
            outp = ctx.enter_context(tc.tile_pool(name="outp", bufs=2))
            ptp = ctx.enter_context(tc.tile_pool(name="ptp", bufs=2, space="PSUM"))
            pop = ctx.enter_context(tc.tile_pool(name="pop", bufs=3, space="PSUM"))

            wt_sb = const.tile([2 * C, C], bf16)
            nc.sync.dma_start(wt_sb[:], wt_d[:])
            b_sb = const.tile([C, 1], f32)
            nc.sync.dma_start(b_sb[:], b_d[:])
            id_sb = const.tile([P, P], bf16)
            nc.sync.dma_start(id_sb[:], id_d[:])
            if GATHER_NIDX:
                nix_sb = const.tile([P, NB * NIF], i16)
                nc.sync.dma_start(nix_sb[:], nix_d[:])
            else:
                nix32_sb = const.tile([P, TILES], i32)
                nc.sync.dma_start(nix32_sb[:], nix32_d[:])
            tix_sb = const.tile([P, NB * LT], i32)
            nc.sync.dma_start(tix_sb[:], tix_d[:])

            swdge_pos = 0
            for b in range(NB):
                rows = min(BK, N + 1 - b * BK)
                catN = catp.tile([P, TBK, P], bf16, tag="catN")
                catT = ctwp.tile([P, LT, P], bf16, tag="catT")
                if GATHER_NIDX:
                    rem = gather_rows[b]
                    col = 0
                    row0 = 0
                    while rem > 0:
                        s = min(GATHER_NIDX, rem)
                        assert s % P == 0
                        nc.gpsimd.dma_gather(
                            out_ap=catN[:, row0:row0 + s // P, :],
                            in_ap=x_d[b * BK:b * BK + rows, :],
                            idxs_ap=nix_sb[:, b * NIF + col:
                                           b * NIF + col + s // 16],
                            num_idxs=s,
                            num_idxs_reg=s,
                            elem_size=P)
                        rem -= s
                        col += s // 16
                        row0 += s // P
                    for k in range(lt_tiles[b]):
                        j = b * LT + k
                        nc.gpsimd.indirect_dma_start(
                            out=catT[:, k, :], out_offset=None, in_=x_d[:],
                            in_offset=bass.IndirectOffsetOnAxis(
                                ap=tix_sb[:, j:j + 1], axis=0))
                else:
                    for k in range(TBK):
                        t = b * TBK + k
                        nc.gpsimd.indirect_dma_start(
                            out=catN[:, k, :], out_offset=None, in_=x_d[:],
                            in_offset=bass.IndirectOffsetOnAxis(
                                ap=nix32_sb[:, t:t + 1], axis=0))
                    for k in range(LT):
                        j = b * LT + k
                        nc.gpsimd.indirect_dma_start(
                            out=catT[:, k, :], out_offset=None, in_=x_d[:],
                            in_offset=bass.IndirectOffsetOnAxis(
                                ap=tix_sb[:, j:j + 1], axis=0))

                ot = outp.tile([C, TBK * P], bf16, tag="ot")
                for g in range(NGRP):
                    gt = min(G, TBK - g * G)
                    pt = ptp.tile([P, G * P], bf16, tag="pt")
                    for kk in range(gt):
                        t = g * G + kk
                        nc.tensor.transpose(
                            out=pt[:, kk * P:(kk + 1) * P],
                            in_=catN[:, t, :], identity=id_sb[:])
                        if t < lt_tiles[b]:
                            nc.tensor.transpose(
                                out=pt[C:P, kk * P:(kk + 1) * P],
                                in_=catT[:, t, 0:C], identity=id_sb[:])
                    ct = ctp.tile([P, G * P], bf16, tag="ct")
                    nc.vector.tensor_copy(ct[:, :gt * P], pt[:, :gt * P])
                    po = pop.tile([C, G * P], f32, tag="po")
                    nc.tensor.matmul(out=po[:, :gt * P], lhsT=wt_sb[:],
                                     rhs=ct[:, :gt * P], start=True, stop=True)
                    nc.scalar.activation(
                        ot[:, g * G * P:g * G * P + gt * P], po[:, :gt * P],
                        mybir.ActivationFunctionType.Relu,
                        bias=b_sb[:, 0:1])
                nc.sync.dma_start(
                    out_d[:, b * TBK * P:(b + 1) * TBK * P], ot[:])

    nc.compile()
    return nc


def _get_compiled(gather_rows=None, lt_tiles=None):
    global _COMPILED
    if _COMPILED is None:
        _COMPILED = _build(gather_rows, lt_tiles)
    return _COMPILED


def _prep_core(nloc, tloc, eid_base):
    """Bucket/sort one core's edges; build device index tables.

    nloc/tloc: [EPC] int32 next / twin(redirected, N=dead) indices.
    Returns (nix [128, NB*NIF] i16, tix [128, NB*LT] i32, eid [SLOTS] i64).
    """
    live = tloc != N
    nb = nloc >> 15
    order = np.lexsort((~live, nb))
    sn = nloc[order]
    st = tloc[order]
    seid = eid_base + order.astype(np.int64)
    counts = np.bincount(nb, minlength=NB)

    nix = np.zeros((P, NB * NIF), np.int16)
    nix32 = np.zeros((P, TILES), np.int32)
    tix = np.full((P, NB * LT), N, np.int32)
    eid = np.full(SLOTS, -1, np.int64)
    lvs = np.zeros(NB, np.int64)

    off = 0
    for b in range(NB):
        cnt = int(counts[b])
        if cnt > TBK * P:
            raise RuntimeError(f"bucket {b} overflow: {cnt} > {TBK * P}")
        nlive = int(live[order[off:off + cnt]].sum())
        if nlive > LT * P:
            raise RuntimeError(f"bucket {b} live overflow: {nlive} > {LT * P}")
        lvs[b] = nlive
        reqs = np.zeros(TBK * P, np.int32)
        reqs[:cnt] = sn[off:off + cnt] - (b << 15)
        a = reqs.reshape(NIF, 16).T.astype(np.int16)      # [16, NIF]
        for r in range(0, P, 16):   # replicate for every Q7 pair (queues 0-3)
            nix[r:r + 16, b * NIF:(b + 1) * NIF] = a
        gq = np.zeros(TBK * P, np.int32)
        gq[:cnt] = sn[off:off + cnt]
        nix32[:, b * TBK:(b + 1) * TBK] = gq.reshape(TBK, P).T
        tw = np.full(LT * P, N, np.int32)
        m = min(cnt, LT * P)
        tw[:m] = st[off:off + m]
        tix[:, b * LT:(b + 1) * LT] = tw.reshape(LT, P).T
        eid[b * TBK * P:b * TBK * P + cnt] = seid[off:off + cnt]
        off += cnt
    return nix, nix32, tix, eid, counts, lvs


def kernel(x, next_idx, twin_idx, has_twin, W, b):
    global LAST_EXEC_NS
    x = np.asarray(x, dtype=np.float32)
    next_idx = np.asarray(next_idx, dtype=np.int32)
    twin_idx = np.asarray(twin_idx, dtype=np.int32)
    has_twin = np.asarray(has_twin)
    W = np.asarray(W, dtype=np.float32)
    b = np.asarray(b, dtype=np.float32)

    trace = bool(os.environ.get("BASS_TRACE"))
    if trace:
        _try_install_ntff_shim()

    # x table: [N+1, 128] bf16 rows = 64 features + 64 zeros; row N all-zero.
    x2 = np.zeros((N + 1, P), bfnp)
    x2[:N, :C] = x.astype(bfnp)
    npad = np.zeros(NPAD, np.int32)
    npad[:N] = next_idx
    npad[N:] = (np.arange(NPAD - N, dtype=np.int64) * 65537 % N).astype(np.int32)
    tpad = np.full(NPAD, N, np.int32)
    tpad[:N] = np.where(has_twin, twin_idx, N).astype(np.int32)

    wt = np.ascontiguousarray(W.T).astype(bfnp)         # [128, 64]
    bias = np.ascontiguousarray(b.reshape(C, 1))        # [64, 1] f32
    ident = np.eye(P, dtype=np.float32).astype(bfnp)

    in_maps = []
    eids = []
    maxc = np.zeros(NB, np.int64)
    maxlv = np.zeros(NB, np.int64)
    for c in range(NCORES):
        sl = slice(c * EPC, (c + 1) * EPC)
        nix, nix32, tix, eid, counts, lvs = _prep_core(
            npad[sl], tpad[sl], c * EPC)
        eids.append(eid)
        maxc = np.maximum(maxc, counts)
        maxlv = np.maximum(maxlv, lvs)
        in_maps.append({"x2": x2, "nix": nix, "nix32": nix32, "tix": tix,
                        "wt": wt, "bias": bias, "ident": ident})

    grows = [min(TBK * P, int(-(-m // P)) * P) for m in maxc]
    lts = [min(LT, int(-(-m // P))) for m in maxlv]
    nc = _get_compiled(grows, lts)
    res = bass_utils.run_bass_kernel_spmd(
        nc, in_maps, core_ids=list(range(NCORES)), trace=trace)
    LAST_EXEC_NS = res.exec_time_ns

    out = np.empty((N, C), np.float32)
    for c in range(NCORES):
        arr = res.results[c]["out"].T.astype(np.float32)   # [SLOTS, 64]
        eid = eids[c]
        m = (eid >= 0) & (eid < N)
        out[eid[m]] = arr[m]
    return out
